# revision 23
# baseline (speedup 1.0000x reference)
"""Fused LSTM cell on 8 Trainium2 NeuronCores.

Data-parallel over the batch: each core handles 1024 of the 8192 rows.
Per core, the two GEMMs (x @ Wx.T + h @ Wh.T) are fused into one
[2048]-contraction GEMM, with the gate nonlinearities + state update
fused into the PSUM eviction path.

Performance structure:
- Host pre-packs activations and weights into the exact SBUF layout,
  so every DMA is a few large contiguous runs per partition instead of
  thousands of 512B descriptors.
- Startup is chunked: tile 0's weights are gate-major so the first
  gate's 512KB lands first on the SP queue, then the activation
  k-chunk pairs stream split across both HWDGE queues; the PE starts
  right after the framework preamble and is only briefly paced by the
  `a` stream during the first gate sweep.
- The SP (sync) queue carries nothing but weights after startup, so
  each tile's 2MB weight prefetch is never stuck behind an output DMA
  waiting on an epilogue semaphore. All small/latency DMAs (bias, c,
  outputs) ride the Activation queue.
- Gate order per tile is [tanh-gate, i, f, o] and the epilogue is
  emitted per-gate as its PSUM banks complete, so the post-last-matmul
  critical chain is just sigmoid(o) -> mult -> DMA instead of the
  whole 7-op gate chain. No SWDGE, which keeps the final drain short.
- Matmul operands are bf16 (fp32r optional via LSTM_VARIANT=f32r):
  same PE rate, half the DMA bytes; accumulation stays fp32 in PSUM.
  Steady-state matmul cadence measured at 216ns per 512-row matmul
  (~99% of the 2.4GHz PE peak), with LDWEIGHTS fully hidden.
"""

import os
import sys
import types

import ml_dtypes
import numpy as np

import concourse.bacc as bacc
import concourse.mybir as mybir
import concourse.tile as tile
from concourse.bass_utils import run_bass_kernel_spmd


def _install_ntff_shim():
    """This image's `antenv` package lacks the `axon_hooks` module, so
    `run_bass_kernel_spmd(trace=True)` would crash on import (and boot()
    skipped registering the NTFF profile hook). Recreate the module and
    register the ctypes hook so tracing works when requested; without
    this the kernel still runs fine as long as nothing asks for a trace.
    """
    try:
        import antenv.axon_hooks  # noqa: F401  (real module exists: done)

        return
    except ImportError:
        pass
    mod = types.ModuleType("antenv.axon_hooks")
    mod._hook = None
    mod.set_axon_ntff_profile_hook = lambda h: setattr(mod, "_hook", h)
    mod.get_axon_ntff_profile_hook = lambda: mod._hook
    sys.modules["antenv.axon_hooks"] = mod
    try:
        import antenv

        antenv.axon_hooks = mod
        from trn_agent_boot.trn_boot import _ntff_profile_via_ctypes

        hook = _ntff_profile_via_ctypes("/opt/axon/libaxon_pjrt.so")
        if hook is not None:
            mod._hook = hook
    except Exception:
        pass


_install_ntff_shim()

B, I, H = 8192, 1024, 1024
NCORES = 8
BL = B // NCORES        # batch rows per core
G4 = 4 * H              # stacked gate dim
KC = (I + H) // 128     # contraction chunks of 128
HT = H // 128           # h-tiles per core
NBC = 2                 # batch chunks per h-tile
BCW = BL // NBC         # 512 columns per matmul (one PSUM bank)

F32 = mybir.dt.float32
BF16 = ml_dtypes.bfloat16
AF = mybir.ActivationFunctionType
OP = mybir.AluOpType

# gate completion order: tanh gate (3) first so its tanh read starts
# early; output gate (2) last so only sigmoid(o)*tanh(c) trails the
# final matmul.
GORDER = (3, 0, 1, 2)

_CACHE: dict = {}


def _build(variant="bf16"):
    mm_dt = mybir.dt.float32r if variant == "f32r" else mybir.dt.bfloat16
    nc = bacc.Bacc("TRN2", target_bir_lowering=False, debug=False)
    aT = nc.dram_tensor("a_t", [128, KC * BL], mm_dt, kind="ExternalInput")
    wT = nc.dram_tensor("w_t", [128, HT * KC * 4 * 128], mm_dt,
                        kind="ExternalInput")
    cT = nc.dram_tensor("c_t", [H, BL], F32, kind="ExternalInput")
    bias = nc.dram_tensor("bias", [128, 4 * HT], F32, kind="ExternalInput")
    cO = nc.dram_tensor("c_out", [H, BL], F32, kind="ExternalOutput")
    hO = nc.dram_tensor("h_out", [H, BL], F32, kind="ExternalOutput")

    a_view = aT.rearrange("p (c b) -> p c b", c=KC)
    w_view = wT.rearrange("p (t c g j) -> p t c g j", t=HT, c=KC, g=4)
    c_view = cT.rearrange("(t p) b -> p t b", p=128)

    with tile.TileContext(nc) as tc:
        with (
            tc.tile_pool(name="resident", bufs=1) as res_pool,
            tc.tile_pool(name="wpool", bufs=2) as w_pool,
            tc.tile_pool(name="cpool", bufs=2) as c_pool,
            tc.tile_pool(name="opool", bufs=2) as o_pool,
            tc.tile_pool(name="act", bufs=3) as act_pool,
            tc.tile_pool(name="psum", bufs=1, space="PSUM") as psum_pool,
        ):
            # ---- startup choreography -------------------------------
            # SP queue:   w0 c-group | a pair | w0 c-group | ... | w1..w7
            # Act queue:  bias | a even pairs | c0 | per-tile c/outs
            # Tile 0 runs its matmuls k-chunk-outer so each arriving
            # a-chunk is consumed by 8 matmuls immediately; the PE then
            # outpaces neither queue and never stalls on the a stream.
            w0_sb = w_pool.tile([128, KC, 4, 128], mm_dt, tag="w0", bufs=1)
            bias_sb = res_pool.tile([128, 4 * HT], F32)
            nc.scalar.dma_start(bias_sb[:], bias[:])
            a_sb = res_pool.tile([128, KC, BL], mm_dt)
            # sync: small early items so the first matmul's (coalesced)
            # queue-semaphore wait clears as early as possible; scalar
            # carries the later a-chunks as quads.
            nc.sync.dma_start(w0_sb[:, 0:2], w_view[:, 0, 0:2])
            nc.sync.dma_start(a_sb[:, 0:2], a_view[:, 0:2])
            nc.sync.dma_start(w0_sb[:, 2:4], w_view[:, 0, 2:4])
            nc.sync.dma_start(a_sb[:, 2:4], a_view[:, 2:4])
            for q in range(3):
                asl = slice(4 * q + 4, 4 * q + 8)
                nc.scalar.dma_start(a_sb[:, asl], a_view[:, asl])
            nc.sync.dma_start(w0_sb[:, 4:8], w_view[:, 0, 4:8])
            nc.sync.dma_start(w0_sb[:, 8:12], w_view[:, 0, 8:12])
            nc.sync.dma_start(w0_sb[:, 12:16], w_view[:, 0, 12:16])

            for t in range(HT):
                if t == 0:
                    w_sb = w0_sb
                else:
                    w_sb = w_pool.tile([128, KC, 4, 128], mm_dt, tag="w")
                    nc.sync.dma_start(w_sb[:], w_view[:, t])

                if t % 2 == 0:
                    cp_sb = c_pool.tile([128, 2, BL], F32, tag="cprev")
                    nc.scalar.dma_start(cp_sb[:], c_view[:, t:t + 2])
                oc_sb = o_pool.tile([128, BL], F32, tag="oc")
                oh_sb = o_pool.tile([128, BL], F32, tag="oh")

                ps = {}
                for g in range(4):
                    for bc in range(NBC):
                        ps[g, bc] = psum_pool.tile(
                            [128, BCW], F32, tag=f"ps{g}{bc}",
                            name=f"ps{g}{bc}", bufs=1)
                ep = {}
                for bc in range(NBC):
                    for nm in ("si", "sf", "so", "tg", "t1", "t2", "tct"):
                        ep[nm, bc] = act_pool.tile([128, BCW], F32,
                                                   tag=f"{nm}{bc}",
                                                   name=f"{nm}{bc}")

                def bias_ap(g):
                    return bias_sb[:, g * HT + t:g * HT + t + 1]

                if t == 0:
                    # k-chunk-outer: all 8 banks accumulate together so
                    # each a-chunk is consumed as soon as it arrives
                    for c in range(KC):
                        for g in GORDER:
                            for bc in range(NBC):
                                bsl = slice(bc * BCW, (bc + 1) * BCW)
                                nc.tensor.matmul(
                                    ps[g, bc][:], w_sb[:, c, g, :],
                                    a_sb[:, c, bsl],
                                    start=(c == 0), stop=(c == KC - 1),
                                )

                for g in GORDER:
                    if t != 0:
                        # (c, bc) ordering: both bc matmuls share a
                        # stationary tile
                        for c in range(KC):
                            for bc in range(NBC):
                                bsl = slice(bc * BCW, (bc + 1) * BCW)
                                nc.tensor.matmul(
                                    ps[g, bc][:], w_sb[:, c, g, :],
                                    a_sb[:, c, bsl],
                                    start=(c == 0), stop=(c == KC - 1),
                                )
                    # emit the epilogue ops that become ready once this
                    # gate's banks stop — they overlap the next gates'
                    # matmuls and release PSUM banks early
                    for bc in range(NBC):
                        bsl = slice(bc * BCW, (bc + 1) * BCW)
                        if g == 3:
                            nc.scalar.activation(ep["tg", bc][:], ps[3, bc][:],
                                                 AF.Tanh, bias=bias_ap(3))
                        elif g == 0:
                            nc.scalar.activation(ep["si", bc][:], ps[0, bc][:],
                                                 AF.Sigmoid, bias=bias_ap(0))
                            nc.vector.tensor_tensor(
                                ep["t2", bc][:], ep["si", bc][:],
                                ep["tg", bc][:], OP.mult)
                        elif g == 1:
                            nc.scalar.activation(ep["sf", bc][:], ps[1, bc][:],
                                                 AF.Sigmoid, bias=bias_ap(1))
                            nc.vector.tensor_tensor(
                                ep["t1", bc][:], ep["sf", bc][:],
                                cp_sb[:, t % 2, bsl], OP.mult)
                            nc.vector.tensor_tensor(
                                oc_sb[:, bsl], ep["t1", bc][:],
                                ep["t2", bc][:], OP.add)
                            nc.scalar.activation(ep["tct", bc][:],
                                                 oc_sb[:, bsl], AF.Tanh)
                            # last tile: flush per-bc so only the final
                            # half-tile trails the last matmul
                            if t == HT - 1:
                                nc.scalar.dma_start(
                                    cO[t * 128:(t + 1) * 128, bsl],
                                    oc_sb[:, bsl])
                            elif bc == NBC - 1:
                                nc.scalar.dma_start(
                                    cO[t * 128:(t + 1) * 128, :], oc_sb[:])
                        else:  # g == 2
                            nc.scalar.activation(ep["so", bc][:], ps[2, bc][:],
                                                 AF.Sigmoid, bias=bias_ap(2))
                            nc.vector.tensor_tensor(
                                oh_sb[:, bsl], ep["so", bc][:],
                                ep["tct", bc][:], OP.mult)
                            if t == HT - 1:
                                # idle sync queue: final h_out transfers
                                # run concurrently with the c_out ones
                                nc.sync.dma_start(
                                    hO[t * 128:(t + 1) * 128, bsl],
                                    oh_sb[:, bsl])
                            elif bc == NBC - 1:
                                nc.scalar.dma_start(
                                    hO[t * 128:(t + 1) * 128, :], oh_sb[:])

    nc.finalize()
    return nc


def _variant() -> str:
    return os.environ.get("LSTM_VARIANT", "bf16")


def kernel(x_current, c_previous, h_previous, Wx, bx, Wh, bh):
    variant = _variant()
    x = np.asarray(x_current, dtype=np.float32)
    c = np.asarray(c_previous, dtype=np.float32)
    h = np.asarray(h_previous, dtype=np.float32)
    Wx = np.asarray(Wx, dtype=np.float32)
    Wh = np.asarray(Wh, dtype=np.float32)
    bsum = np.asarray(bx, dtype=np.float32) + np.asarray(bh, dtype=np.float32)

    mm_np = np.float32 if variant == "f32r" else BF16

    # [4H, I+H] -> per-tile SBUF blocks [c, g, j] per partition;
    # contiguous 16KB-per-partition DMAs
    W = np.concatenate([Wx, Wh], axis=1)
    w5 = W.reshape(4, HT, 128, KC, 128).transpose(4, 1, 3, 0, 2)  # p t c g j
    w_prep = np.ascontiguousarray(w5).astype(mm_np).reshape(
        128, HT * KC * 4 * 128)
    bias_t = np.ascontiguousarray(bsum.reshape(4 * HT, 128).T)  # [128, 32]

    in_maps = []
    for core in range(NCORES):
        sl = slice(core * BL, (core + 1) * BL)
        A = np.concatenate([x[sl], h[sl]], axis=1)  # [BL, 2048]
        a_prep = np.ascontiguousarray(
            A.reshape(BL, KC, 128).transpose(2, 1, 0)
        ).astype(mm_np).reshape(128, KC * BL)
        in_maps.append({
            "a_t": a_prep,
            "w_t": w_prep,
            "c_t": np.ascontiguousarray(c[sl].T),
            "bias": bias_t,
        })

    key = f"nc_{variant}"
    if key not in _CACHE:
        _CACHE[key] = _build(variant)
    nc = _CACHE[key]

    res = run_bass_kernel_spmd(
        nc, in_maps, list(range(NCORES)),
        trace=bool(int(os.environ.get("LSTM_TRACE", "0"))),
    )
    _CACHE["last_result"] = res

    c_out = np.empty((B, H), dtype=np.float32)
    h_out = np.empty((B, H), dtype=np.float32)
    for core in range(NCORES):
        sl = slice(core * BL, (core + 1) * BL)
        c_out[sl] = res.results[core]["c_out"].T
        h_out[sl] = res.results[core]["h_out"].T
    return c_out, h_out


# revision 24
# speedup vs baseline: 1.0230x; 1.0230x over previous
"""Fused LSTM cell on 8 Trainium2 NeuronCores.

Data-parallel over the batch: each core handles 1024 of the 8192 rows.
Per core, the two GEMMs (x @ Wx.T + h @ Wh.T) are fused into one
[2048]-contraction GEMM, with the gate nonlinearities + state update
fused into the PSUM eviction path.

Performance structure:
- Host pre-packs activations and weights into the exact SBUF layout,
  so every DMA is a few large contiguous runs per partition instead of
  thousands of 512B descriptors.
- Startup is chunked: tile 0's weights are gate-major so the first
  gate's 512KB lands first on the SP queue, then the activation
  k-chunk pairs stream split across both HWDGE queues; the PE starts
  right after the framework preamble and is only briefly paced by the
  `a` stream during the first gate sweep.
- The SP (sync) queue carries nothing but weights after startup, so
  each tile's 2MB weight prefetch is never stuck behind an output DMA
  waiting on an epilogue semaphore. All small/latency DMAs (bias, c,
  outputs) ride the Activation queue.
- Gate order per tile is [tanh-gate, i, f, o] and the epilogue is
  emitted per-gate as its PSUM banks complete, so the post-last-matmul
  critical chain is just sigmoid(o) -> mult -> DMA instead of the
  whole 7-op gate chain. No SWDGE, which keeps the final drain short.
- Matmul operands are bf16 (fp32r optional via LSTM_VARIANT=f32r):
  same PE rate, half the DMA bytes; accumulation stays fp32 in PSUM.
  Steady-state matmul cadence measured at 216ns per 512-row matmul
  (~99% of the 2.4GHz PE peak), with LDWEIGHTS fully hidden.
"""

import os
import sys
import types

import ml_dtypes
import numpy as np

import concourse.bacc as bacc
import concourse.mybir as mybir
import concourse.tile as tile
from concourse.bass_utils import run_bass_kernel_spmd


def _install_ntff_shim():
    """This image's `antenv` package lacks the `axon_hooks` module, so
    `run_bass_kernel_spmd(trace=True)` would crash on import (and boot()
    skipped registering the NTFF profile hook). Recreate the module and
    register the ctypes hook so tracing works when requested; without
    this the kernel still runs fine as long as nothing asks for a trace.
    """
    try:
        import antenv.axon_hooks  # noqa: F401  (real module exists: done)

        return
    except ImportError:
        pass
    mod = types.ModuleType("antenv.axon_hooks")
    mod._hook = None
    mod.set_axon_ntff_profile_hook = lambda h: setattr(mod, "_hook", h)
    mod.get_axon_ntff_profile_hook = lambda: mod._hook
    sys.modules["antenv.axon_hooks"] = mod
    try:
        import antenv

        antenv.axon_hooks = mod
        from trn_agent_boot.trn_boot import _ntff_profile_via_ctypes

        hook = _ntff_profile_via_ctypes("/opt/axon/libaxon_pjrt.so")
        if hook is not None:
            mod._hook = hook
    except Exception:
        pass


_install_ntff_shim()

B, I, H = 8192, 1024, 1024
NCORES = 8
BL = B // NCORES        # batch rows per core
G4 = 4 * H              # stacked gate dim
KC = (I + H) // 128     # contraction chunks of 128
HT = H // 128           # h-tiles per core
NBC = 2                 # batch chunks per h-tile
BCW = BL // NBC         # 512 columns per matmul (one PSUM bank)

F32 = mybir.dt.float32
BF16 = ml_dtypes.bfloat16
AF = mybir.ActivationFunctionType
OP = mybir.AluOpType

# gate completion order: tanh gate (3) first so its tanh read starts
# early; output gate (2) last so only sigmoid(o)*tanh(c) trails the
# final matmul.
GORDER = (3, 0, 1, 2)

_CACHE: dict = {}


def _build(variant="bf16"):
    mm_dt = mybir.dt.float32r if variant == "f32r" else mybir.dt.bfloat16
    nc = bacc.Bacc("TRN2", target_bir_lowering=False, debug=False)
    aT = nc.dram_tensor("a_t", [128, KC * BL], mm_dt, kind="ExternalInput")
    wT = nc.dram_tensor("w_t", [128, HT * KC * 4 * 128], mm_dt,
                        kind="ExternalInput")
    cT = nc.dram_tensor("c_t", [H, BL], F32, kind="ExternalInput")
    bias = nc.dram_tensor("bias", [128, 4 * HT], F32, kind="ExternalInput")
    cO = nc.dram_tensor("c_out", [H, BL], F32, kind="ExternalOutput")
    hO = nc.dram_tensor("h_out", [H, BL], F32, kind="ExternalOutput")

    a_view = aT.rearrange("p (c b) -> p c b", c=KC)
    w_view = wT.rearrange("p (t c g j) -> p t c g j", t=HT, c=KC, g=4)
    c_view = cT.rearrange("(t p) b -> p t b", p=128)

    with tile.TileContext(nc) as tc:
        with (
            tc.tile_pool(name="resident", bufs=1) as res_pool,
            tc.tile_pool(name="wpool", bufs=2) as w_pool,
            tc.tile_pool(name="cpool", bufs=2) as c_pool,
            tc.tile_pool(name="opool", bufs=2) as o_pool,
            tc.tile_pool(name="act", bufs=3) as act_pool,
            tc.tile_pool(name="psum", bufs=1, space="PSUM") as psum_pool,
        ):
            # ---- startup choreography -------------------------------
            # SP queue:   w0 c-group | a pair | w0 c-group | ... | w1..w7
            # Act queue:  bias | a even pairs | c0 | per-tile c/outs
            # Tile 0 runs its matmuls k-chunk-outer so each arriving
            # a-chunk is consumed by 8 matmuls immediately; the PE then
            # outpaces neither queue and never stalls on the a stream.
            w0_sb = w_pool.tile([128, KC, 4, 128], mm_dt, tag="w0", bufs=1)
            bias_sb = res_pool.tile([128, 4 * HT], F32)
            nc.scalar.dma_start(bias_sb[:], bias[:])
            a_sb = res_pool.tile([128, KC, BL], mm_dt)
            # interleave weight-groups and a-pairs across both queues:
            # fine granularity keeps the (coalesced) queue-semaphore
            # waits of the first matmuls clearing just ahead of use
            for q in range(4):
                csl = slice(q * 4, (q + 1) * 4)
                nc.sync.dma_start(w0_sb[:, csl], w_view[:, 0, csl])
                lo = slice(q * 4, q * 4 + 2)
                hi = slice(q * 4 + 2, q * 4 + 4)
                nc.sync.dma_start(a_sb[:, lo], a_view[:, lo])
                nc.scalar.dma_start(a_sb[:, hi], a_view[:, hi])

            for t in range(HT):
                if t == 0:
                    w_sb = w0_sb
                else:
                    w_sb = w_pool.tile([128, KC, 4, 128], mm_dt, tag="w")
                    nc.sync.dma_start(w_sb[:], w_view[:, t])

                if t % 2 == 0:
                    cp_sb = c_pool.tile([128, 2, BL], F32, tag="cprev")
                    nc.scalar.dma_start(cp_sb[:], c_view[:, t:t + 2])
                oc_sb = o_pool.tile([128, BL], F32, tag="oc")
                oh_sb = o_pool.tile([128, BL], F32, tag="oh")

                ps = {}
                for g in range(4):
                    for bc in range(NBC):
                        ps[g, bc] = psum_pool.tile(
                            [128, BCW], F32, tag=f"ps{g}{bc}",
                            name=f"ps{g}{bc}", bufs=1)
                ep = {}
                for bc in range(NBC):
                    for nm in ("si", "sf", "so", "tg", "t1", "t2", "tct"):
                        ep[nm, bc] = act_pool.tile([128, BCW], F32,
                                                   tag=f"{nm}{bc}",
                                                   name=f"{nm}{bc}")

                def bias_ap(g):
                    return bias_sb[:, g * HT + t:g * HT + t + 1]

                if t == 0:
                    # k-chunk-outer: all 8 banks accumulate together so
                    # each a-chunk is consumed as soon as it arrives
                    for c in range(KC):
                        for g in GORDER:
                            for bc in range(NBC):
                                bsl = slice(bc * BCW, (bc + 1) * BCW)
                                nc.tensor.matmul(
                                    ps[g, bc][:], w_sb[:, c, g, :],
                                    a_sb[:, c, bsl],
                                    start=(c == 0), stop=(c == KC - 1),
                                )

                for g in GORDER:
                    if t != 0:
                        # (c, bc) ordering: both bc matmuls share a
                        # stationary tile
                        for c in range(KC):
                            for bc in range(NBC):
                                bsl = slice(bc * BCW, (bc + 1) * BCW)
                                nc.tensor.matmul(
                                    ps[g, bc][:], w_sb[:, c, g, :],
                                    a_sb[:, c, bsl],
                                    start=(c == 0), stop=(c == KC - 1),
                                )
                    # emit the epilogue ops that become ready once this
                    # gate's banks stop — they overlap the next gates'
                    # matmuls and release PSUM banks early
                    for bc in range(NBC):
                        bsl = slice(bc * BCW, (bc + 1) * BCW)
                        if g == 3:
                            nc.scalar.activation(ep["tg", bc][:], ps[3, bc][:],
                                                 AF.Tanh, bias=bias_ap(3))
                        elif g == 0:
                            nc.scalar.activation(ep["si", bc][:], ps[0, bc][:],
                                                 AF.Sigmoid, bias=bias_ap(0))
                            nc.vector.tensor_tensor(
                                ep["t2", bc][:], ep["si", bc][:],
                                ep["tg", bc][:], OP.mult)
                        elif g == 1:
                            nc.scalar.activation(ep["sf", bc][:], ps[1, bc][:],
                                                 AF.Sigmoid, bias=bias_ap(1))
                            nc.vector.tensor_tensor(
                                ep["t1", bc][:], ep["sf", bc][:],
                                cp_sb[:, t % 2, bsl], OP.mult)
                            nc.vector.tensor_tensor(
                                oc_sb[:, bsl], ep["t1", bc][:],
                                ep["t2", bc][:], OP.add)
                            nc.scalar.activation(ep["tct", bc][:],
                                                 oc_sb[:, bsl], AF.Tanh)
                            # last tile: flush per-bc so only the final
                            # half-tile trails the last matmul
                            if t == HT - 1:
                                nc.scalar.dma_start(
                                    cO[t * 128:(t + 1) * 128, bsl],
                                    oc_sb[:, bsl])
                            elif bc == NBC - 1:
                                nc.scalar.dma_start(
                                    cO[t * 128:(t + 1) * 128, :], oc_sb[:])
                        else:  # g == 2
                            nc.scalar.activation(ep["so", bc][:], ps[2, bc][:],
                                                 AF.Sigmoid, bias=bias_ap(2))
                            nc.vector.tensor_tensor(
                                oh_sb[:, bsl], ep["so", bc][:],
                                ep["tct", bc][:], OP.mult)
                            if t == HT - 1:
                                # idle sync queue: final h_out transfers
                                # run concurrently with the c_out ones
                                nc.sync.dma_start(
                                    hO[t * 128:(t + 1) * 128, bsl],
                                    oh_sb[:, bsl])
                            elif bc == NBC - 1:
                                nc.scalar.dma_start(
                                    hO[t * 128:(t + 1) * 128, :], oh_sb[:])

    nc.finalize()
    return nc


def _variant() -> str:
    return os.environ.get("LSTM_VARIANT", "bf16")


def kernel(x_current, c_previous, h_previous, Wx, bx, Wh, bh):
    variant = _variant()
    x = np.asarray(x_current, dtype=np.float32)
    c = np.asarray(c_previous, dtype=np.float32)
    h = np.asarray(h_previous, dtype=np.float32)
    Wx = np.asarray(Wx, dtype=np.float32)
    Wh = np.asarray(Wh, dtype=np.float32)
    bsum = np.asarray(bx, dtype=np.float32) + np.asarray(bh, dtype=np.float32)

    mm_np = np.float32 if variant == "f32r" else BF16

    # [4H, I+H] -> per-tile SBUF blocks [c, g, j] per partition;
    # contiguous 16KB-per-partition DMAs
    W = np.concatenate([Wx, Wh], axis=1)
    w5 = W.reshape(4, HT, 128, KC, 128).transpose(4, 1, 3, 0, 2)  # p t c g j
    w_prep = np.ascontiguousarray(w5).astype(mm_np).reshape(
        128, HT * KC * 4 * 128)
    bias_t = np.ascontiguousarray(bsum.reshape(4 * HT, 128).T)  # [128, 32]

    in_maps = []
    for core in range(NCORES):
        sl = slice(core * BL, (core + 1) * BL)
        A = np.concatenate([x[sl], h[sl]], axis=1)  # [BL, 2048]
        a_prep = np.ascontiguousarray(
            A.reshape(BL, KC, 128).transpose(2, 1, 0)
        ).astype(mm_np).reshape(128, KC * BL)
        in_maps.append({
            "a_t": a_prep,
            "w_t": w_prep,
            "c_t": np.ascontiguousarray(c[sl].T),
            "bias": bias_t,
        })

    key = f"nc_{variant}"
    if key not in _CACHE:
        _CACHE[key] = _build(variant)
    nc = _CACHE[key]

    res = run_bass_kernel_spmd(
        nc, in_maps, list(range(NCORES)),
        trace=bool(int(os.environ.get("LSTM_TRACE", "0"))),
    )
    _CACHE["last_result"] = res

    c_out = np.empty((B, H), dtype=np.float32)
    h_out = np.empty((B, H), dtype=np.float32)
    for core in range(NCORES):
        sl = slice(core * BL, (core + 1) * BL)
        c_out[sl] = res.results[core]["c_out"].T
        h_out[sl] = res.results[core]["h_out"].T
    return c_out, h_out
